# revision 11
# baseline (speedup 1.0000x reference)
"""AttentionConv kernel for Trainium2 (8 NeuronCores, SPMD data-parallel over batch).

Problem: per-channel windowed softmax attention.
  q = wq @ x; k = wk @ pad(x, 3); v = wv @ pad(x, 3)       (1x1 convs = GEMMs)
  s_j[c,w] = q[c,w] * k[c,w+j],  j = 0..6
  out[c,w] = sum_j softmax_j(s)[c,w,j] * v[c,w+j]

Sharding: batch B=8 -> one batch element per core; weights replicated.
Since pad commutes with the channel-mixing GEMM, k/v are computed on the
unpadded x and written into SBUF buffers with 3 zero columns on each side.

Per-core engine mapping:
  TensorE: 3 GEMMs (256x256 @ 256x4096), bf16 in, fp32 PSUM.
  ScalarE: batched PSUM->SBUF evacuation casts, exp, 1/den via exp(-ln(den))
           (Exp+Ln pinned to one ACT table set), bf16->fp32 output upcast.
  VectorE: windowed score mult, e*v mult, tree adds for num/den (all bf16,
           innermost stride 1 -> 2x DVE mode), final bf16 out = num * rden.
"""

import sys

sys.path.insert(0, "/opt/trn_rl_repo")

import numpy as np

B, C, W = 8, 256, 4096
K7, PAD = 7, 3
WC_G = 1024  # gemm / psum evac group (2 PSUM banks)
WC_A = 2048  # attention chunk

_STATE = {}


def _patch_act_tables():
    """Force Exp and Ln to resolve to the one ACT table set containing both,
    so the kernel pays a single ACT_TABLE_LOAD instead of thrashing."""
    import concourse.bacc as bacc_mod
    import concourse.mybir as mybir
    from concourse.hw_specs import get_activation_tables as orig

    AF = mybir.ActivationFunctionType

    def patched(arch):
        out = {}
        for name, funcs in orig(arch).items():
            f = set(funcs)
            if name != "natural_log_exp_and_others":
                f.discard(AF.Exp)
                f.discard(AF.Ln)
            out[name] = f
        return out

    bacc_mod.get_activation_tables = patched


def _build_nc():
    import concourse.bass as bass
    import concourse.tile as tile
    from concourse import bacc, mybir

    _patch_act_tables()

    bf16 = mybir.dt.bfloat16
    f32 = mybir.dt.float32
    AF = mybir.ActivationFunctionType

    nc = bacc.Bacc("TRN2", target_bir_lowering=False, debug=False, num_devices=8)

    x_d = nc.declare_dram_parameter("x", [C, W], bf16, isOutput=False)
    w_d = {
        t: nc.declare_dram_parameter(f"wt{t}", [C, C], bf16, isOutput=False)
        for t in "qkv"
    }
    out_d = nc.declare_dram_parameter("out", [C, W], bf16, isOutput=True)

    WP = W + 2 * PAD  # padded width for k/v
    n_ag = W // WC_G  # gemm groups per co block
    n_ac = W // WC_A  # attention chunks per co block

    with tile.TileContext(nc) as tc:
        from contextlib import ExitStack

        with ExitStack() as ctx:
            persist = ctx.enter_context(tc.tile_pool(name="persist", bufs=1))
            psum = ctx.enter_context(tc.tile_pool(name="psum", bufs=3, space="PSUM"))
            spool = ctx.enter_context(tc.tile_pool(name="spool", bufs=3))
            dpool = ctx.enter_context(tc.tile_pool(name="dpool", bufs=2))
            opool = ctx.enter_context(tc.tile_pool(name="opool", bufs=2))

            # ---- persistent SBUF tensors ----
            xb = persist.tile([128, 2, W], bf16, tag="xb")  # x, ci-major blocks
            wsb = {
                t: persist.tile([128, 2, C], bf16, name=f"wsb_{t}", tag=f"wsb_{t}")
                for t in "qkv"
            }  # w.T
            qsb = persist.tile([128, 2, W], bf16, tag="qsb")
            ksb = persist.tile([128, 2, WP], bf16, tag="ksb")
            vsb = persist.tile([128, 2, WP], bf16, tag="vsb")

            # ---- loads ----
            # wq first (feeds the PE warmup), then x (critical path), then wk/wv.
            for cb in range(2):
                nc.sync.dma_start(
                    out=wsb["q"][:, cb, :], in_=w_d["q"][cb * 128 : (cb + 1) * 128, :]
                )
            for cb in range(2):
                nc.sync.dma_start(
                    out=xb[:, cb, :], in_=x_d[cb * 128 : (cb + 1) * 128, :]
                )
            for t in "kv":
                for cb in range(2):
                    nc.sync.dma_start(
                        out=wsb[t][:, cb, :], in_=w_d[t][cb * 128 : (cb + 1) * 128, :]
                    )

            # zero the pad columns of k and v
            for buf in (ksb, vsb):
                for cb in range(2):
                    nc.vector.memset(buf[:, cb, 0:PAD], 0.0)
                    nc.vector.memset(buf[:, cb, W + PAD : WP], 0.0)

            # PE warmup burst: ~7us of dummy matmuls on the wq tiles so the
            # HAM clock-gate releases before the real GEMM stream arrives.
            wps = psum.tile([128, WC_G], f32, name="wps", tag="ps")
            for i in range(28):
                nc.tensor.matmul(
                    wps[:, 0:256],
                    wsb["q"][:, 0, 0:128],
                    wsb["q"][:, i % 2, :],
                    start=True,
                    stop=True,
                    skip_group_check=True,
                )

            def gemm_group(co, g):
                """q/k/v GEMM for output cols [g*WC_G, (g+1)*WC_G) of co-block,
                batched into one PSUM tile + one ACT evacuation per tensor."""
                co_sl = slice(co * 128, (co + 1) * 128)
                for t in "qkv":
                    ps = psum.tile([128, WC_G], f32, name="ps", tag="ps")
                    for i in range(WC_G // 512):
                        w0 = g * WC_G + i * 512
                        for ci in range(2):
                            nc.tensor.matmul(
                                ps[:, i * 512 : (i + 1) * 512],
                                wsb[t][:, ci, co_sl],
                                xb[:, ci, w0 : w0 + 512],
                                start=(ci == 0),
                                stop=(ci == 1),
                            )
                    if t == "q":
                        dst = qsb[:, co, g * WC_G : (g + 1) * WC_G]
                    else:
                        buf = ksb if t == "k" else vsb
                        dst = buf[:, co, PAD + g * WC_G : PAD + (g + 1) * WC_G]
                    nc.scalar.copy(out=dst, in_=ps[:, :])

            def att_scores(co, ai):
                """scores + exp for att chunk (co, ai); returns the e tile.
                Split into j-halves so exp starts before all scores finish."""
                w0 = ai * WC_A
                s = spool.tile([128, K7, WC_A], bf16, name="s", tag="s")

                qsl = qsb[:, co, w0 : w0 + WC_A]
                ksl = ksb[:, co, w0 : w0 + WC_A]

                def q_bc(n):
                    return bass.AP(
                        tensor=qsl.tensor,
                        offset=qsl.offset,
                        ap=[qsl.ap[0], [0, n], [1, WC_A]],
                    )

                def k_wn(j0, n):
                    return bass.AP(
                        tensor=ksl.tensor,
                        offset=ksl.offset + j0,
                        ap=[ksl.ap[0], [1, n], [1, WC_A]],
                    )

                # scores then e = exp(s) in place, in two j-halves
                nc.vector.tensor_mul(s[:, 0:4, :], q_bc(4), k_wn(0, 4))
                nc.scalar.activation(s[:, 0:4, :], s[:, 0:4, :], AF.Exp)
                nc.vector.tensor_mul(s[:, 4:7, :], q_bc(3), k_wn(4, 3))
                nc.scalar.activation(s[:, 4:7, :], s[:, 4:7, :], AF.Exp)
                return s

            def att_rest(co, ai, s):
                """softmax-normalize + v-window weighted sum + store."""
                w0 = ai * WC_A
                co_sl = slice(co * 128, (co + 1) * 128)
                dent = dpool.tile([128, 3, WC_A], bf16, name="dent", tag="dent")
                vsl = vsb[:, co, w0 : w0 + WC_A]
                v_w = bass.AP(
                    tensor=vsl.tensor,
                    offset=vsl.offset,
                    ap=[vsl.ap[0], [1, K7], [1, WC_A]],
                )
                # den tree -> dent[:, 0, :]
                # pairs (e0+e1, e2+e3) need only the first exp half
                s02 = bass.AP(
                    tensor=s.tensor, offset=s.offset,
                    ap=[s.ap[0], [2 * WC_A, 2], [1, WC_A]],
                )
                s13 = bass.AP(
                    tensor=s.tensor, offset=s.offset + WC_A,
                    ap=[s.ap[0], [2 * WC_A, 2], [1, WC_A]],
                )
                nc.vector.tensor_add(dent[:, 0:2, :], s02, s13)
                nc.vector.tensor_add(dent[:, 2, :], s[:, 4, :], s[:, 5, :])
                nc.vector.tensor_add(dent[:, 0, :], dent[:, 0, :], dent[:, 1, :])
                nc.vector.tensor_add(dent[:, 0, :], dent[:, 0, :], dent[:, 2, :])
                nc.vector.tensor_add(dent[:, 0, :], dent[:, 0, :], s[:, 6, :])
                # rden = exp(-ln(den)) -> dent[:, 1, :]
                nc.scalar.activation(dent[:, 1, :], dent[:, 0, :], AF.Ln)
                nc.scalar.activation(dent[:, 1, :], dent[:, 1, :], AF.Exp, scale=-1.0)
                # ev = e * v_shift, in place; num tree -> s[:, 0, :]
                nc.vector.tensor_mul(s[:, :, :], s[:, :, :], v_w)
                nc.vector.tensor_add(s[:, 0:3, :], s[:, 0:3, :], s[:, 3:6, :])
                nc.vector.tensor_add(s[:, 0, :], s[:, 0, :], s[:, 1, :])
                nc.vector.tensor_add(s[:, 0, :], s[:, 0, :], s[:, 2, :])
                nc.vector.tensor_add(s[:, 0, :], s[:, 0, :], s[:, 6, :])
                # out = num * rden (bf16, 2x mode); host upcasts to fp32
                oc = opool.tile([128, WC_A], bf16, name="oc", tag="oc")
                nc.vector.tensor_mul(oc[:, :], s[:, 0, :], dent[:, 1, :])
                nc.sync.dma_start(out=out_d[co_sl, w0 : w0 + WC_A], in_=oc[:, :])

            # Software-pipelined emission: engines run their streams in order,
            # so chunk i+1's scores must be emitted before chunk i's tail or
            # the DVE stalls behind ACT's exp.
            gpg = WC_A // WC_G  # gemm groups per attention chunk
            chunks = [(co, ai) for co in range(2) for ai in range(n_ac)]
            emitted = [0, 0]  # gemm groups emitted per co block

            def need_gemms(co, ai):
                hi = min((ai + 1) * gpg + 1, n_ag)
                for g in range(emitted[co], hi):
                    gemm_group(co, g)
                emitted[co] = max(emitted[co], hi)

            import os
            if os.environ.get("KPIPE", "1") == "1":
                need_gemms(*chunks[0])
                tiles = {chunks[0]: att_scores(*chunks[0])}
                for idx, ch in enumerate(chunks):
                    if idx + 1 < len(chunks):
                        nxt = chunks[idx + 1]
                        need_gemms(*nxt)
                        tiles[nxt] = att_scores(*nxt)
                    att_rest(*ch, tiles.pop(ch))
            else:
                for ch in chunks:
                    need_gemms(*ch)
                    att_rest(*ch, att_scores(*ch))

    nc.finalize()
    return nc


def _get_nc():
    if "nc" not in _STATE:
        _STATE["nc"] = _build_nc()
    return _STATE["nc"]


def kernel(x, wq, wk, wv):
    import ml_dtypes

    bf = ml_dtypes.bfloat16
    nc = _get_nc()

    x = np.asarray(x, dtype=np.float32)
    wqT = np.ascontiguousarray(np.asarray(wq, dtype=np.float32).T).astype(bf)
    wkT = np.ascontiguousarray(np.asarray(wk, dtype=np.float32).T).astype(bf)
    wvT = np.ascontiguousarray(np.asarray(wv, dtype=np.float32).T).astype(bf)
    xb = x.astype(bf)

    in_maps = [
        {
            "x": np.ascontiguousarray(xb[b]),
            "wtq": wqT,
            "wtk": wkT,
            "wtv": wvT,
        }
        for b in range(B)
    ]

    from concourse.bass_utils import run_bass_kernel_spmd

    res = run_bass_kernel_spmd(nc, in_maps, core_ids=list(range(B)))
    outs = [np.asarray(res.results[i]["out"], dtype=np.float32) for i in range(B)]
    return np.stack(outs)
